# revision 21
# baseline (speedup 1.0000x reference)
"""Differential attention kernel for TRN2, 8 NeuronCores.

Problem: q,k,v [2, 2048, 8, 64] f32; out [2, 8, 1024, 64]:
  S = (Q @ K^T) / 8 per (b,h); P = softmax(S); out = (P[:1024] - lam*P[1024:]) @ V
  lam = exp(lq1.lk1) - exp(lq2.lk2) + LAMBDA_INIT

Sharding: 16 (b,h) slabs, 2 per core. Per slab, on-device:
  - host pre-stages bf16 layouts: q^T duplicated into both partition halves,
    k^T row-packed (even j-tiles on partitions 0-63, odd on 64-127), and
    V''=[V|1] in [128, NT, 65] tile layout (denominator via ones column)
  - S^T chunks via PAIRED bf16 matmuls on disjoint PE row groups (K=64 each,
    tile_position (0,0)/(64,0) inferred from base partitions) so the two
    j-tiles of a pair stream concurrently through the array
  - ACT exp (scale=1/8 folded) over [128,1024] PSUM -> bf16 SBUF
  - PV: Otilde^T accumulated in PSUM with V'' stationary, es moving (bf16)
  - bf16 PE-transposes of Otilde^T back to row-major, DVE normalize rows +
    lam-combine halves, DMA out

Emission is software-pipelined: next slab's loads early, PV lags exp by one
j-pair, epilogues of half ih are spread through the next half's loop.
"""

import math
import sys

sys.path.insert(0, "/opt/trn_rl_repo")

import numpy as np

B, N, H, D = 2, 2048, 8, 64
P = 128
NT = N // P  # 16 k-tiles per slab
NP = NT // 2  # 8 k-tile pairs
SLABS_PER_CORE = 2
N_CORES = 8
LAMBDA_INIT = 0.8 - 0.6 * math.exp(-0.3 * 0.8)

# Schraudolph fast-exp on DVE for a subset of chunks (ACT is the bottleneck
# engine): exp(s/8) ~= bitcast_bf16(int16(s * A_EFF + B_SCH)) — bf16 is the
# top half of f32 so the classic trick works at 2^7 mantissa scale, and the
# result feeds the bf16 PV matmuls directly (the backend rejects mixed
# 32/16-bit matmul inputs). C=7.375 centers the piecewise-linear 2^f
# approximation for N(0,1) logits (~1.8% RMS weight error on the offloaded
# fraction; end-to-end rel err stays < 1e-2 vs the 2e-2 gate).
A_SCH = 2.0**7 / math.log(2.0)
A_EFF = A_SCH / 8.0
B_SCH = float(127 * 2**7) - 7.375
# (jp, c) chunks computed on DVE per query-half; jp0 stays on ACT (it is
# emitted in the rotated prologue/tail position). 3 of 16 chunks balances
# ACT relief against DVE load across both burst and sustained clock regimes
# (measured best-of {0,2,3,5} on HW).
DVE_SET = frozenset({(1, 0), (3, 1), (6, 0)})

_cached_nc = {}


def _build_program(repeats=1):
    """Build the Bass program. `repeats` wraps the computation in an on-device
    loop (identical results; used only for slope-based HW timing)."""
    if repeats in _cached_nc:
        return _cached_nc[repeats]

    import concourse.mybir as mybir
    import concourse.tile as tile
    from concourse import bacc
    from concourse.masks import make_identity

    f32 = mybir.dt.float32
    f32r = mybir.dt.float32r
    bf16 = mybir.dt.bfloat16
    i16 = mybir.dt.int16
    AF = mybir.ActivationFunctionType
    ALU = mybir.AluOpType

    nc = bacc.Bacc("TRN2", target_bir_lowering=False, debug=False)
    qs = nc.dram_tensor("qs", [SLABS_PER_CORE, P, N], bf16, kind="ExternalInput").ap()
    ks = nc.dram_tensor(
        "ks", [SLABS_PER_CORE, P, NP, P], bf16, kind="ExternalInput"
    ).ap()
    vs = nc.dram_tensor(
        "vs", [SLABS_PER_CORE, P, NT, D + 1], bf16, kind="ExternalInput"
    ).ap()
    lams = nc.dram_tensor("lams", [1, 4 * D], f32, kind="ExternalInput").ap()
    out = nc.dram_tensor(
        "out", [SLABS_PER_CORE, N // 2, D], f32, kind="ExternalOutput"
    ).ap()

    with tile.TileContext(nc) as tc:
        with (
            tc.tile_pool(name="const", bufs=1) as cpool,
            tc.tile_pool(name="inp", bufs=2) as inpool,
            tc.tile_pool(name="es", bufs=4) as espool,
            tc.tile_pool(name="esd", bufs=4) as esdpool,
            tc.tile_pool(name="osb", bufs=4) as osbpool,
            tc.tile_pool(name="fin", bufs=2) as finpool,
            tc.tile_pool(name="ptr", bufs=2, space="PSUM") as ptrpool,
            tc.tile_pool(name="pst", bufs=2, space="PSUM") as pstpool,
            tc.tile_pool(name="pot", bufs=2, space="PSUM") as potpool,
        ):
            identb = cpool.tile([D + 1, D + 1], bf16)
            make_identity(nc, identb[:])
            lamb = cpool.tile([P, 1], f32)

            def emit_lambda():
                # DMA on the (otherwise idle) SWDGE ring so the SP ring stays
                # free for the head-critical K/Q loads
                lt = cpool.tile([1, 4 * D], f32)
                nc.gpsimd.dma_start(lt[:], lams)
                prod = cpool.tile([1, D], f32)
                lam2 = cpool.tile([1, 2], f32)
                nc.vector.tensor_mul(prod[:], lt[:, 0:D], lt[:, D : 2 * D])
                nc.vector.reduce_sum(lam2[:, 0:1], prod[:], axis=mybir.AxisListType.X)
                nc.vector.tensor_mul(
                    prod[:], lt[:, 2 * D : 3 * D], lt[:, 3 * D : 4 * D]
                )
                nc.vector.reduce_sum(lam2[:, 1:2], prod[:], axis=mybir.AxisListType.X)
                elam = cpool.tile([1, 2], f32)
                nc.scalar.activation(elam[:], lam2[:], AF.Exp)
                lfull = cpool.tile([1, 1], f32)
                nc.vector.tensor_sub(lfull[:], elam[:, 0:1], elam[:, 1:2])
                nc.vector.tensor_scalar_add(lfull[:], lfull[:], LAMBDA_INIT)
                ones = cpool.tile([1, P], f32)
                nc.vector.memset(ones[:], 1.0)
                plam = ptrpool.tile([P, 1], f32, tag="ptr")
                nc.tensor.matmul(plam[:], ones[:], lfull[:], start=True, stop=True)
                nc.vector.tensor_copy(lamb[:], plam[:])

            def emit_loads(s_rep):
                """DMA loads for one slab (all layouts prebuilt on host)."""
                s = s_rep % SLABS_PER_CORE
                kf = inpool.tile([P, NP, P], bf16, tag="kf")
                qf = inpool.tile([P, N], bf16, tag="qf")
                vf = inpool.tile([P, NT, D + 1], bf16, tag="vf")
                nc.sync.dma_start(kf[:], ks[s])
                nc.sync.dma_start(qf[:, 0:1024], qs[s][:, 0:1024])
                nc.sync.dma_start(qf[:, 1024:N], qs[s][:, 1024:N])
                nc.sync.dma_start(vf[:], vs[s])
                return kf, qf, vf

            def emit_chunk_epilogue(s_rep, ih, c, ot2, onn):
                """Drain one [65,512] PV chunk: transpose, normalize; for ih1
                also combine with ih0 and DMA out."""
                s = s_rep % SLABS_PER_CORE
                osb = osbpool.tile([D + 1, 512], bf16, tag="osb")
                nc.vector.tensor_copy(osb[:], ot2[c][:])
                pto = ptrpool.tile([P, 4, D + 2], bf16, tag="ptr")
                for u in range(4):
                    nc.tensor.transpose(
                        pto[:, u, 0 : D + 1],
                        osb[:, P * u : P * (u + 1)],
                        identb[:],
                    )
                rec = finpool.tile([P, 4], f32, tag="rec")
                nc.vector.reciprocal(rec[:], pto[:, :, D])
                if ih == 1:
                    nc.vector.tensor_scalar_mul(rec[:], rec[:], lamb[:, 0:1])
                t0 = NT * ih // 2 + 4 * c
                nc.vector.tensor_mul(
                    onn[:, t0 : t0 + 4, :],
                    pto[:, :, 0:D],
                    rec[:].broadcast_to([P, 4, D]),
                )
                if ih == 1:
                    # on the otherwise-idle GPSIMD (SBUF-only operands) to
                    # keep DVE headroom for the Schraudolph chunks
                    dd = finpool.tile([P, 4, D], f32, tag="dd")
                    nc.gpsimd.tensor_sub(
                        dd[:], onn[:, 4 * c : 4 * c + 4, :], onn[:, t0 : t0 + 4, :]
                    )
                    nc.sync.dma_start(
                        out[s].rearrange("(t p) d -> p t d", p=P)[:, 4 * c : 4 * c + 4, :],
                        dd[:],
                    )

            def emit_qk(kf, qf, ih, jp):
                """QK^T for one j-pair: 4 paired bf16 matmuls on disjoint PE
                row groups (0-63 / 64-127 concurrent) + exp per chunk — on
                ACT (exact), or on DVE via Schraudolph for DVE_SET chunks."""
                es = espool.tile([P, 2048], bf16, tag="es")
                srcs = [None, None]
                for c in range(2):
                    st = pstpool.tile([P, 1024], f32, tag="pst")
                    nc.tensor.matmul(
                        st[:, 0:512],
                        kf[0:D, jp, :],
                        qf[0:D, 1024 * ih + 512 * c : 1024 * ih + 512 * (c + 1)],
                        start=True,
                        stop=True,
                    )
                    nc.tensor.matmul(
                        st[:, 512:1024],
                        kf[D:P, jp, :],
                        qf[D:P, 1024 * ih + 512 * c : 1024 * ih + 512 * (c + 1)],
                        start=True,
                        stop=True,
                    )
                    if (jp, c) in DVE_SET:
                        esd = esdpool.tile([P, 1024], i16, tag="esd")
                        nc.vector.tensor_scalar(
                            esd[:], st[:], A_EFF, B_SCH, ALU.mult, ALU.add
                        )
                        srcs[c] = esd[:].bitcast(bf16)
                    else:
                        nc.scalar.activation(
                            es[:, 1024 * c : 1024 * (c + 1)],
                            st[:],
                            AF.Exp,
                            scale=1.0 / 8.0,
                        )
                        srcs[c] = es[:, 1024 * c : 1024 * (c + 1)]
                return srcs

            def emit_half(s_rep, kf, qf, vf, ih, onn, fillers, pending,
                          tail=None):
                """One query-half loop over j-pairs starting at jp1 — jp0's
                QK^T/exp was emitted inside the previous half (or prologue)
                and arrives via `pending`. The PREVIOUS pair's 4 PV matmuls
                lag the exp by one step; `fillers` are emitted one per step.
                `tail` (emitted between the last exp and the last PV) emits
                the NEXT half's jp0 QK^T/exp so the ACT stream never breaks
                at half boundaries."""
                ot2 = [
                    potpool.tile([D + 1, 512], f32, tag="pot", name=f"ot_{ih}_{c}")
                    for c in range(2)
                ]

                def emit_pv(jp, srcs):
                    j0, j1 = 2 * jp, 2 * jp + 1
                    first = jp == 0
                    last = jp == NP - 1
                    for c in range(2):
                        nc.tensor.matmul(
                            ot2[c][:],
                            vf[:, j0, :],
                            srcs[c][:, 0:512],
                            start=first,
                            stop=False,
                        )
                        nc.tensor.matmul(
                            ot2[c][:],
                            vf[:, j1, :],
                            srcs[c][:, 512:1024],
                            start=False,
                            stop=last,
                        )

                for jp in range(1, NP):
                    srcs = emit_qk(kf, qf, ih, jp)
                    emit_pv(*pending)
                    pending = (jp, srcs)
                    if fillers:
                        fillers.pop(0)()
                if tail is not None:
                    tail()
                emit_pv(*pending)
                return [
                    (lambda c=c: emit_chunk_epilogue(s_rep, ih, c, ot2, onn))
                    for c in range(2)
                ]

            # ---- software-pipelined emission across slabs AND iterations ---
            # Prologue (once): slab0 loads + lambda + slab0's first QK^T/exp.
            # Body: rotated modulo the iteration — slab0-ih0 starts at jp1
            # consuming the pending pair; the tail emits the NEXT iteration's
            # first QK^T/exp (into the same ring slots as the prologue's, so
            # addresses are iteration-invariant) BEFORE draining the last
            # epilogues, keeping ACT fed across the loop-wrap.
            def emit_body(last):
                def make_tail(tiles_ref, ih):
                    def tail():
                        kfn, qfn, _ = tiles_ref[0]
                        pend_qk[0] = (0, emit_qk(kfn, qfn, ih, 0))

                    return tail

                kf, qf, vf = tiles[0]
                cur = [(kf, qf, vf)]
                onn0 = osbpool.tile([P, NT, D], f32, tag="onn")
                epi0 = emit_half(0, kf, qf, vf, 0, onn0, [], pend_qk[0],
                                 tail=make_tail(cur, 1))
                tiles[0] = emit_loads(1)
                epi1 = emit_half(0, kf, qf, vf, 1, onn0, epi0, pend_qk[0],
                                 tail=make_tail(tiles, 0))
                kf1, qf1, vf1 = tiles[0]
                cur1 = [(kf1, qf1, vf1)]
                onn1 = osbpool.tile([P, NT, D], f32, tag="onn")
                epi0b = emit_half(1, kf1, qf1, vf1, 0, onn1, epi1, pend_qk[0],
                                  tail=make_tail(cur1, 1))
                tiles[0] = emit_loads(2)
                epi1b = emit_half(1, kf1, qf1, vf1, 1, onn1, epi0b, pend_qk[0],
                                  tail=None if last else make_tail(tiles, 0))
                for thunk in epi1b:
                    thunk()

            tiles = [emit_loads(0)]
            emit_lambda()
            kf0, qf0, vf0 = tiles[0]
            pend_qk = [(0, emit_qk(kf0, qf0, 0, 0))]
            if repeats == 1:
                emit_body(last=True)
            else:
                with tc.For_i(0, repeats, 1):
                    emit_body(last=False)

    nc.compile()
    _cached_nc[repeats] = nc
    return nc


def staged_in_maps(inputs):
    """FULL host inputs -> per-core input dicts (the sharding + layouts)."""
    import ml_dtypes

    bf16 = ml_dtypes.bfloat16

    q = np.asarray(inputs["q"], dtype=np.float32)
    k = np.asarray(inputs["k"], dtype=np.float32)
    v = np.asarray(inputs["v"], dtype=np.float32)
    lams = np.concatenate(
        [
            np.asarray(inputs["lambda_q1"], dtype=np.float32),
            np.asarray(inputs["lambda_k1"], dtype=np.float32),
            np.asarray(inputs["lambda_q2"], dtype=np.float32),
            np.asarray(inputs["lambda_k2"], dtype=np.float32),
        ]
    ).reshape(1, 4 * D)

    # [b, n, h, d] -> slabs, b-major
    qT = q.transpose(0, 2, 3, 1).reshape(B * H, D, N).astype(bf16)
    kT = k.transpose(0, 2, 3, 1).reshape(B * H, D, N).astype(bf16)
    vso = v.transpose(0, 2, 1, 3).reshape(B * H, N, D).astype(bf16)

    # q^T duplicated into both partition halves: [S, 128, N]
    qs = np.concatenate([qT, qT], axis=1)
    # k^T row-packed: even j-tiles on partitions 0-63, odd on 64-127
    kt3 = kT.reshape(B * H, D, NT, P)
    ks = np.concatenate([kt3[:, :, 0::2, :], kt3[:, :, 1::2, :]], axis=1)
    # V'' = [V | 1] in [S, 128, NT, 65] tile layout
    vt = vso.reshape(B * H, NT, P, D).transpose(0, 2, 1, 3)
    ones = np.ones((B * H, P, NT, 1), dtype=bf16)
    vsp = np.concatenate([vt, ones], axis=3)

    return [
        {
            "qs": np.ascontiguousarray(qs[SLABS_PER_CORE * c : SLABS_PER_CORE * (c + 1)]),
            "ks": np.ascontiguousarray(ks[SLABS_PER_CORE * c : SLABS_PER_CORE * (c + 1)]),
            "vs": np.ascontiguousarray(vsp[SLABS_PER_CORE * c : SLABS_PER_CORE * (c + 1)]),
            "lams": lams,
        }
        for c in range(N_CORES)
    ]


def unshard_out(stacked):
    """[n_cores, *per-core out shape] -> [b, h, n/2, d] fp32."""
    return np.asarray(stacked).reshape(B, H, N // 2, D).astype(np.float32)


def kernel(q, k, v, lambda_q1, lambda_k1, lambda_q2, lambda_k2, **_unused):
    from concourse.bass_utils import run_bass_kernel_spmd

    in_maps = staged_in_maps(
        dict(q=q, k=k, v=v, lambda_q1=lambda_q1, lambda_k1=lambda_k1,
             lambda_q2=lambda_q2, lambda_k2=lambda_k2)
    )
    nc = _build_program()
    res = run_bass_kernel_spmd(nc, in_maps, core_ids=list(range(N_CORES)))
    outs = np.stack([res.results[c]["out"] for c in range(N_CORES)])
    return unshard_out(outs)


# revision 26
# speedup vs baseline: 1.2999x; 1.2999x over previous
"""Differential attention kernel for TRN2, 8 NeuronCores.

Problem: q,k,v [2, 2048, 8, 64] f32; out [2, 8, 1024, 64]:
  S = (Q @ K^T) / 8 per (b,h); P = softmax(S); out = (P[:1024] - lam*P[1024:]) @ V
  lam = exp(lq1.lk1) - exp(lq2.lk2) + LAMBDA_INIT

Sharding: 16 (b,h) slabs, 2 per core. Per slab, on-device:
  - host pre-stages bf16 layouts: q^T duplicated into both partition halves,
    k^T row-packed (even j-tiles on partitions 0-63, odd on 64-127), and
    V''=[V|1] in [128, NT, 65] tile layout (denominator via ones column)
  - S^T chunks via PAIRED bf16 matmuls on disjoint PE row groups (K=64 each,
    tile_position (0,0)/(64,0) inferred from base partitions) so the two
    j-tiles of a pair stream concurrently through the array
  - ACT exp (scale=1/8 folded) over [128,1024] PSUM -> bf16 SBUF
  - PV: Otilde^T accumulated in PSUM with V'' stationary, es moving (bf16)
  - bf16 PE-transposes of Otilde^T back to row-major, DVE normalize rows +
    lam-combine halves, DMA out

Emission is software-pipelined: next slab's loads early, PV lags exp by one
j-pair, epilogues of half ih are spread through the next half's loop.
"""

import math
import sys

sys.path.insert(0, "/opt/trn_rl_repo")

import numpy as np

B, N, H, D = 2, 2048, 8, 64
P = 128
NT = N // P  # 16 k-tiles per slab
NP = NT // 2  # 8 k-tile pairs
SLABS_PER_CORE = 2
N_CORES = 8
LAMBDA_INIT = 0.8 - 0.6 * math.exp(-0.3 * 0.8)

# Schraudolph fast-exp on DVE for a subset of chunks (ACT is the bottleneck
# engine): exp(s/8) ~= bitcast_bf16(int16(s * A_EFF + B_SCH)) — bf16 is the
# top half of f32 so the classic trick works at 2^7 mantissa scale, and the
# result feeds the bf16 PV matmuls directly (the backend rejects mixed
# 32/16-bit matmul inputs). C=7.375 centers the piecewise-linear 2^f
# approximation for N(0,1) logits (~1.8% RMS weight error on the offloaded
# fraction; end-to-end rel err stays < 1e-2 vs the 2e-2 gate).
A_SCH = 2.0**7 / math.log(2.0)
A_EFF = A_SCH / 8.0
B_SCH = float(127 * 2**7) - 7.375
# (jp, c) chunks computed on DVE per query-half; jp0 stays on ACT (it is
# emitted in the rotated prologue/tail position). 3 of 16 chunks balances
# ACT relief against DVE load across both burst and sustained clock regimes
# (measured best-of {0,2,3,5} on HW).
DVE_SET = frozenset({(1, 0), (3, 1), (6, 0)})

_cached_nc = {}


def _build_program(repeats=1):
    """Build the Bass program. `repeats` wraps the computation in an on-device
    loop (identical results; used only for slope-based HW timing)."""
    if repeats in _cached_nc:
        return _cached_nc[repeats]

    import concourse.mybir as mybir
    import concourse.tile as tile
    from concourse import bacc
    from concourse.masks import make_identity

    f32 = mybir.dt.float32
    f32r = mybir.dt.float32r
    bf16 = mybir.dt.bfloat16
    i16 = mybir.dt.int16
    AF = mybir.ActivationFunctionType
    ALU = mybir.AluOpType

    nc = bacc.Bacc("TRN2", target_bir_lowering=False, debug=False)
    qs = nc.dram_tensor("qs", [SLABS_PER_CORE, P, N], bf16, kind="ExternalInput").ap()
    ks = nc.dram_tensor(
        "ks", [SLABS_PER_CORE, P, NP, P], bf16, kind="ExternalInput"
    ).ap()
    vs = nc.dram_tensor(
        "vs", [SLABS_PER_CORE, P, NT, D + 1], bf16, kind="ExternalInput"
    ).ap()
    lams = nc.dram_tensor("lams", [P, 4 * D], f32, kind="ExternalInput").ap()
    out = nc.dram_tensor(
        "out", [SLABS_PER_CORE, N // 2, D], f32, kind="ExternalOutput"
    ).ap()

    with tile.TileContext(nc) as tc:
        with (
            tc.tile_pool(name="const", bufs=1) as cpool,
            tc.tile_pool(name="inp", bufs=2) as inpool,
            tc.tile_pool(name="es", bufs=4) as espool,
            tc.tile_pool(name="esd", bufs=4) as esdpool,
            tc.tile_pool(name="osb", bufs=4) as osbpool,
            tc.tile_pool(name="fin", bufs=2) as finpool,
            tc.tile_pool(name="pst", bufs=2, space="PSUM") as pstpool,
            tc.tile_pool(name="pot", bufs=4, space="PSUM") as potpool,
        ):
            identb = cpool.tile([D + 1, D + 1], bf16)
            make_identity(nc, identb[:])
            lamb = cpool.tile([P, 1], f32)

            def emit_lambda():
                # lams host-tiled to all 128 partitions: the tiny lambda
                # reduction runs per-partition redundantly (same cost — DVE
                # charges free-size) and lands in lamb [P,1] directly, with
                # no PE broadcast matmul and no PSUM tile.
                # DMA on the (otherwise idle) SWDGE ring so the SP ring stays
                # free for the head-critical K/Q loads
                lt = cpool.tile([P, 4 * D], f32)
                nc.gpsimd.dma_start(lt[:], lams)
                prod = cpool.tile([P, D], f32)
                lam2 = cpool.tile([P, 2], f32)
                nc.vector.tensor_mul(prod[:], lt[:, 0:D], lt[:, D : 2 * D])
                nc.vector.reduce_sum(lam2[:, 0:1], prod[:], axis=mybir.AxisListType.X)
                nc.vector.tensor_mul(
                    prod[:], lt[:, 2 * D : 3 * D], lt[:, 3 * D : 4 * D]
                )
                nc.vector.reduce_sum(lam2[:, 1:2], prod[:], axis=mybir.AxisListType.X)
                elam = cpool.tile([P, 2], f32)
                nc.scalar.activation(elam[:], lam2[:], AF.Exp)
                nc.vector.tensor_sub(lamb[:], elam[:, 0:1], elam[:, 1:2])
                nc.vector.tensor_scalar_add(lamb[:], lamb[:], LAMBDA_INIT)

            def emit_loads(s_rep):
                """DMA loads for one slab (all layouts prebuilt on host)."""
                s = s_rep % SLABS_PER_CORE
                kf = inpool.tile([P, NP, P], bf16, tag="kf")
                qf = inpool.tile([P, N], bf16, tag="qf")
                vf = inpool.tile([P, NT, D + 1], bf16, tag="vf")
                nc.sync.dma_start(kf[:], ks[s])
                nc.sync.dma_start(qf[:, 0:1024], qs[s][:, 0:1024])
                nc.sync.dma_start(qf[:, 1024:N], qs[s][:, 1024:N])
                nc.sync.dma_start(vf[:], vs[s])
                return kf, qf, vf

            def emit_chunk_epilogue(s_rep, ih, c, ot2, onn):
                """Drain one [65,512] PV chunk: transpose, normalize; for ih1
                also combine with ih0 and DMA out."""
                s = s_rep % SLABS_PER_CORE
                osb = osbpool.tile([D + 1, 512], bf16, tag="osb")
                nc.vector.tensor_copy(osb[:], ot2[c][:])
                # pto shares the pot ring: each slot is freed by the osb
                # drain that immediately precedes the transposes
                pto = potpool.tile([P, 4, D + 2], bf16, tag="pot")
                for u in range(4):
                    nc.tensor.transpose(
                        pto[:, u, 0 : D + 1],
                        osb[:, P * u : P * (u + 1)],
                        identb[:],
                    )
                rec = finpool.tile([P, 4], f32, tag="rec")
                nc.vector.reciprocal(rec[:], pto[:, :, D])
                if ih == 1:
                    nc.vector.tensor_scalar_mul(rec[:], rec[:], lamb[:, 0:1])
                t0 = NT * ih // 2 + 4 * c
                nc.vector.tensor_mul(
                    onn[:, t0 : t0 + 4, :],
                    pto[:, :, 0:D],
                    rec[:].broadcast_to([P, 4, D]),
                )
                if ih == 1:
                    # on the otherwise-idle GPSIMD (SBUF-only operands) to
                    # keep DVE headroom for the Schraudolph chunks
                    dd = finpool.tile([P, 4, D], f32, tag="dd")
                    nc.gpsimd.tensor_sub(
                        dd[:], onn[:, 4 * c : 4 * c + 4, :], onn[:, t0 : t0 + 4, :]
                    )
                    nc.sync.dma_start(
                        out[s].rearrange("(t p) d -> p t d", p=P)[:, 4 * c : 4 * c + 4, :],
                        dd[:],
                    )

            def emit_qk(kf, qf, ih, jp):
                """QK^T for one j-pair: 4 paired bf16 matmuls on disjoint PE
                row groups (0-63 / 64-127 concurrent) + exp per chunk — on
                ACT (exact), or on DVE via Schraudolph for DVE_SET chunks."""
                es = espool.tile([P, 2048], bf16, tag="es")
                srcs = [None, None]
                for c in range(2):
                    st = pstpool.tile([P, 1024], f32, tag="pst")
                    nc.tensor.matmul(
                        st[:, 0:512],
                        kf[0:D, jp, :],
                        qf[0:D, 1024 * ih + 512 * c : 1024 * ih + 512 * (c + 1)],
                        start=True,
                        stop=True,
                    )
                    nc.tensor.matmul(
                        st[:, 512:1024],
                        kf[D:P, jp, :],
                        qf[D:P, 1024 * ih + 512 * c : 1024 * ih + 512 * (c + 1)],
                        start=True,
                        stop=True,
                    )
                    if (jp, c) in DVE_SET:
                        esd = esdpool.tile([P, 1024], i16, tag="esd")
                        nc.vector.tensor_scalar(
                            esd[:], st[:], A_EFF, B_SCH, ALU.mult, ALU.add
                        )
                        srcs[c] = esd[:].bitcast(bf16)
                    else:
                        nc.scalar.activation(
                            es[:, 1024 * c : 1024 * (c + 1)],
                            st[:],
                            AF.Exp,
                            scale=1.0 / 8.0,
                        )
                        srcs[c] = es[:, 1024 * c : 1024 * (c + 1)]
                return srcs

            def emit_half(s_rep, kf, qf, vf, ih, onn, fillers, pending,
                          tail=None):
                """One query-half loop over j-pairs starting at jp1 — jp0's
                QK^T/exp was emitted inside the previous half (or prologue)
                and arrives via `pending`. The PREVIOUS pair's 4 PV matmuls
                lag the exp by one step; `fillers` are emitted one per step.
                `tail` (emitted between the last exp and the last PV) emits
                the NEXT half's jp0 QK^T/exp so the ACT stream never breaks
                at half boundaries."""
                ot2 = [
                    potpool.tile([D + 1, 512], f32, tag="pot", name=f"ot_{ih}_{c}")
                    for c in range(2)
                ]

                def emit_pv(jp, srcs):
                    j0, j1 = 2 * jp, 2 * jp + 1
                    first = jp == 0
                    last = jp == NP - 1
                    for c in range(2):
                        nc.tensor.matmul(
                            ot2[c][:],
                            vf[:, j0, :],
                            srcs[c][:, 0:512],
                            start=first,
                            stop=False,
                        )
                        nc.tensor.matmul(
                            ot2[c][:],
                            vf[:, j1, :],
                            srcs[c][:, 512:1024],
                            start=False,
                            stop=last,
                        )

                for jp in range(1, NP):
                    srcs = emit_qk(kf, qf, ih, jp)
                    emit_pv(*pending)
                    pending = (jp, srcs)
                    if fillers:
                        fillers.pop(0)()
                if tail is not None:
                    tail()
                emit_pv(*pending)
                return [
                    (lambda c=c: emit_chunk_epilogue(s_rep, ih, c, ot2, onn))
                    for c in range(2)
                ]

            # ---- software-pipelined emission across slabs AND iterations ---
            # Prologue (once): slab0 loads + lambda + slab0's first QK^T/exp.
            # Body: rotated modulo the iteration — slab0-ih0 starts at jp1
            # consuming the pending pair; the tail emits the NEXT iteration's
            # first QK^T/exp (into the same ring slots as the prologue's, so
            # addresses are iteration-invariant) BEFORE draining the last
            # epilogues, keeping ACT fed across the loop-wrap.
            def emit_body(last):
                def make_tail(tiles_ref, ih):
                    def tail():
                        kfn, qfn, _ = tiles_ref[0]
                        pend_qk[0] = (0, emit_qk(kfn, qfn, ih, 0))

                    return tail

                kf, qf, vf = tiles[0]
                cur = [(kf, qf, vf)]
                onn0 = osbpool.tile([P, NT, D], f32, tag="onn")
                epi0 = emit_half(0, kf, qf, vf, 0, onn0, [], pend_qk[0],
                                 tail=make_tail(cur, 1))
                tiles[0] = emit_loads(1)
                epi1 = emit_half(0, kf, qf, vf, 1, onn0, epi0, pend_qk[0],
                                 tail=make_tail(tiles, 0))
                kf1, qf1, vf1 = tiles[0]
                cur1 = [(kf1, qf1, vf1)]
                onn1 = osbpool.tile([P, NT, D], f32, tag="onn")
                epi0b = emit_half(1, kf1, qf1, vf1, 0, onn1, epi1, pend_qk[0],
                                  tail=make_tail(cur1, 1))
                tiles[0] = emit_loads(2)
                epi1b = emit_half(1, kf1, qf1, vf1, 1, onn1, epi0b, pend_qk[0],
                                  tail=None if last else make_tail(tiles, 0))
                for thunk in epi1b:
                    thunk()

            tiles = [emit_loads(0)]
            emit_lambda()
            kf0, qf0, vf0 = tiles[0]
            pend_qk = [(0, emit_qk(kf0, qf0, 0, 0))]
            if repeats == 1:
                emit_body(last=True)
            else:
                with tc.For_i(0, repeats, 1):
                    emit_body(last=False)

    nc.compile()
    _cached_nc[repeats] = nc
    return nc


def staged_in_maps(inputs):
    """FULL host inputs -> per-core input dicts (the sharding + layouts)."""
    import ml_dtypes

    bf16 = ml_dtypes.bfloat16

    q = np.asarray(inputs["q"], dtype=np.float32)
    k = np.asarray(inputs["k"], dtype=np.float32)
    v = np.asarray(inputs["v"], dtype=np.float32)
    lams = np.concatenate(
        [
            np.asarray(inputs["lambda_q1"], dtype=np.float32),
            np.asarray(inputs["lambda_k1"], dtype=np.float32),
            np.asarray(inputs["lambda_q2"], dtype=np.float32),
            np.asarray(inputs["lambda_k2"], dtype=np.float32),
        ]
    ).reshape(1, 4 * D)
    # tiled to all partitions so lambda is computed per-partition on device
    lams = np.ascontiguousarray(np.broadcast_to(lams, (P, 4 * D)))

    # [b, n, h, d] -> slabs, b-major
    qT = q.transpose(0, 2, 3, 1).reshape(B * H, D, N).astype(bf16)
    kT = k.transpose(0, 2, 3, 1).reshape(B * H, D, N).astype(bf16)
    vso = v.transpose(0, 2, 1, 3).reshape(B * H, N, D).astype(bf16)

    # q^T duplicated into both partition halves: [S, 128, N]
    qs = np.concatenate([qT, qT], axis=1)
    # k^T row-packed: even j-tiles on partitions 0-63, odd on 64-127
    kt3 = kT.reshape(B * H, D, NT, P)
    ks = np.concatenate([kt3[:, :, 0::2, :], kt3[:, :, 1::2, :]], axis=1)
    # V'' = [V | 1] in [S, 128, NT, 65] tile layout
    vt = vso.reshape(B * H, NT, P, D).transpose(0, 2, 1, 3)
    ones = np.ones((B * H, P, NT, 1), dtype=bf16)
    vsp = np.concatenate([vt, ones], axis=3)

    return [
        {
            "qs": np.ascontiguousarray(qs[SLABS_PER_CORE * c : SLABS_PER_CORE * (c + 1)]),
            "ks": np.ascontiguousarray(ks[SLABS_PER_CORE * c : SLABS_PER_CORE * (c + 1)]),
            "vs": np.ascontiguousarray(vsp[SLABS_PER_CORE * c : SLABS_PER_CORE * (c + 1)]),
            "lams": lams,
        }
        for c in range(N_CORES)
    ]


def unshard_out(stacked):
    """[n_cores, *per-core out shape] -> [b, h, n/2, d] fp32."""
    return np.asarray(stacked).reshape(B, H, N // 2, D).astype(np.float32)


def kernel(q, k, v, lambda_q1, lambda_k1, lambda_q2, lambda_k2, **_unused):
    from concourse.bass_utils import run_bass_kernel_spmd

    in_maps = staged_in_maps(
        dict(q=q, k=k, v=v, lambda_q1=lambda_q1, lambda_k1=lambda_k1,
             lambda_q2=lambda_q2, lambda_k2=lambda_k2)
    )
    nc = _build_program()
    res = run_bass_kernel_spmd(nc, in_maps, core_ids=list(range(N_CORES)))
    outs = np.stack([res.results[c]["out"] for c in range(N_CORES)])
    return unshard_out(outs)


# revision 28
# speedup vs baseline: 1.5618x; 1.2015x over previous
"""Differential attention kernel for TRN2, 8 NeuronCores.

Problem: q,k,v [2, 2048, 8, 64] f32; out [2, 8, 1024, 64]:
  S = (Q @ K^T) / 8 per (b,h); P = softmax(S); out = (P[:1024] - lam*P[1024:]) @ V
  lam = exp(lq1.lk1) - exp(lq2.lk2) + LAMBDA_INIT

Sharding: 16 (b,h) slabs, 2 per core. Per slab, on-device:
  - host pre-stages bf16 layouts: q^T duplicated into both partition halves,
    k^T row-packed (even j-tiles on partitions 0-63, odd on 64-127), and
    V''=[V|1] in [128, NT, 65] tile layout (denominator via ones column)
  - S^T chunks via PAIRED bf16 matmuls on disjoint PE row groups (K=64 each,
    tile_position (0,0)/(64,0) inferred from base partitions) so the two
    j-tiles of a pair stream concurrently through the array
  - ACT exp (scale=1/8 folded) over [128,1024] PSUM -> bf16 SBUF
  - PV: Otilde^T accumulated in PSUM with V'' stationary, es moving (bf16)
  - bf16 PE-transposes of Otilde^T back to row-major, DVE normalize rows +
    lam-combine halves, DMA out

Emission is software-pipelined: next slab's loads early, PV lags exp by one
j-pair, epilogues of half ih are spread through the next half's loop.
"""

import math
import sys

sys.path.insert(0, "/opt/trn_rl_repo")

import numpy as np

B, N, H, D = 2, 2048, 8, 64
P = 128
NT = N // P  # 16 k-tiles per slab
NP = NT // 2  # 8 k-tile pairs
SLABS_PER_CORE = 2
N_CORES = 8
LAMBDA_INIT = 0.8 - 0.6 * math.exp(-0.3 * 0.8)

# Schraudolph fast-exp on DVE for a subset of chunks (ACT is the bottleneck
# engine): exp(s/8) ~= bitcast_bf16(int16(s * A_EFF + B_SCH)) — bf16 is the
# top half of f32 so the classic trick works at 2^7 mantissa scale, and the
# result feeds the bf16 PV matmuls directly (the backend rejects mixed
# 32/16-bit matmul inputs). C=7.375 centers the piecewise-linear 2^f
# approximation for N(0,1) logits (~1.8% RMS weight error on the offloaded
# fraction; end-to-end rel err stays < 1e-2 vs the 2e-2 gate).
A_SCH = 2.0**7 / math.log(2.0)
A_EFF = A_SCH / 8.0
B_SCH = float(127 * 2**7) - 7.375
# (jp, c) chunks computed on DVE per query-half; jp0 stays on ACT (it is
# emitted in the rotated prologue/tail position). 3 of 16 chunks balances
# ACT relief against DVE load across both burst and sustained clock regimes
# (measured best-of {0,2,3,5} on HW); late pairs avoid contending with the
# epilogue fillers that occupy DVE at steps jp1-jp2.
DVE_SET = frozenset({(3, 0), (5, 1), (7, 0)})

_cached_nc = {}


def _build_program(repeats=1):
    """Build the Bass program. `repeats` wraps the computation in an on-device
    loop (identical results; used only for slope-based HW timing)."""
    if repeats in _cached_nc:
        return _cached_nc[repeats]

    import concourse.mybir as mybir
    import concourse.tile as tile
    from concourse import bacc
    from concourse.masks import make_identity

    f32 = mybir.dt.float32
    f32r = mybir.dt.float32r
    bf16 = mybir.dt.bfloat16
    i16 = mybir.dt.int16
    AF = mybir.ActivationFunctionType
    ALU = mybir.AluOpType

    nc = bacc.Bacc("TRN2", target_bir_lowering=False, debug=False)
    qs = nc.dram_tensor("qs", [SLABS_PER_CORE, P, N], bf16, kind="ExternalInput").ap()
    ks = nc.dram_tensor(
        "ks", [SLABS_PER_CORE, P, NP, P], bf16, kind="ExternalInput"
    ).ap()
    vs = nc.dram_tensor(
        "vs", [SLABS_PER_CORE, P, NT, D + 1], bf16, kind="ExternalInput"
    ).ap()
    lams = nc.dram_tensor("lams", [P, 4 * D], f32, kind="ExternalInput").ap()
    out = nc.dram_tensor(
        "out", [SLABS_PER_CORE, N // 2, D], f32, kind="ExternalOutput"
    ).ap()

    with tile.TileContext(nc) as tc:
        with (
            tc.tile_pool(name="const", bufs=1) as cpool,
            tc.tile_pool(name="inp", bufs=2) as inpool,
            tc.tile_pool(name="es", bufs=8) as espool,
            tc.tile_pool(name="esd", bufs=6) as esdpool,
            tc.tile_pool(name="osb", bufs=4) as osbpool,
            tc.tile_pool(name="fin", bufs=4) as finpool,
            tc.tile_pool(name="pst", bufs=2, space="PSUM") as pstpool,
            tc.tile_pool(name="pot", bufs=4, space="PSUM") as potpool,
        ):
            identb = cpool.tile([D + 1, D + 1], bf16)
            make_identity(nc, identb[:])
            lamb = cpool.tile([P, 1], f32)

            def emit_lambda():
                # lams host-tiled to all 128 partitions: the tiny lambda
                # reduction runs per-partition redundantly (same cost — DVE
                # charges free-size) and lands in lamb [P,1] directly, with
                # no PE broadcast matmul and no PSUM tile.
                # DMA on the (otherwise idle) SWDGE ring so the SP ring stays
                # free for the head-critical K/Q loads
                lt = cpool.tile([P, 4 * D], f32)
                nc.gpsimd.dma_start(lt[:], lams)
                prod = cpool.tile([P, D], f32)
                lam2 = cpool.tile([P, 2], f32)
                nc.vector.tensor_mul(prod[:], lt[:, 0:D], lt[:, D : 2 * D])
                nc.vector.reduce_sum(lam2[:, 0:1], prod[:], axis=mybir.AxisListType.X)
                nc.vector.tensor_mul(
                    prod[:], lt[:, 2 * D : 3 * D], lt[:, 3 * D : 4 * D]
                )
                nc.vector.reduce_sum(lam2[:, 1:2], prod[:], axis=mybir.AxisListType.X)
                elam = cpool.tile([P, 2], f32)
                nc.scalar.activation(elam[:], lam2[:], AF.Exp)
                nc.vector.tensor_sub(lamb[:], elam[:, 0:1], elam[:, 1:2])
                nc.vector.tensor_scalar_add(lamb[:], lamb[:], LAMBDA_INIT)

            def emit_loads(s_rep):
                """DMA loads for one slab (all layouts prebuilt on host)."""
                s = s_rep % SLABS_PER_CORE
                kf = inpool.tile([P, NP, P], bf16, tag="kf")
                qf = inpool.tile([P, N], bf16, tag="qf")
                vf = inpool.tile([P, NT, D + 1], bf16, tag="vf")
                nc.sync.dma_start(kf[:], ks[s])
                nc.sync.dma_start(qf[:, 0:1024], qs[s][:, 0:1024])
                nc.sync.dma_start(qf[:, 1024:N], qs[s][:, 1024:N])
                nc.sync.dma_start(vf[:], vs[s])
                return kf, qf, vf

            def emit_chunk_epilogue(s_rep, ih, c, ot2, onn):
                """Drain one [65,512] PV chunk: transpose, normalize; for ih1
                also combine with ih0 and DMA out."""
                s = s_rep % SLABS_PER_CORE
                osb = osbpool.tile([D + 1, 512], bf16, tag="osb")
                nc.vector.tensor_copy(osb[:], ot2[c][:])
                # pto shares the pot ring: each slot is freed by the osb
                # drain that immediately precedes the transposes
                pto = potpool.tile([P, 4, D + 2], bf16, tag="pot")
                for u in range(4):
                    nc.tensor.transpose(
                        pto[:, u, 0 : D + 1],
                        osb[:, P * u : P * (u + 1)],
                        identb[:],
                    )
                rec = finpool.tile([P, 4], f32, tag="rec")
                nc.vector.reciprocal(rec[:], pto[:, :, D])
                if ih == 1:
                    nc.vector.tensor_scalar_mul(rec[:], rec[:], lamb[:, 0:1])
                t0 = NT * ih // 2 + 4 * c
                nc.vector.tensor_mul(
                    onn[:, t0 : t0 + 4, :],
                    pto[:, :, 0:D],
                    rec[:].broadcast_to([P, 4, D]),
                )
                if ih == 1:
                    # on the otherwise-idle GPSIMD (SBUF-only operands) to
                    # keep DVE headroom for the Schraudolph chunks
                    dd = finpool.tile([P, 4, D], f32, tag="dd")
                    nc.gpsimd.tensor_sub(
                        dd[:], onn[:, 4 * c : 4 * c + 4, :], onn[:, t0 : t0 + 4, :]
                    )
                    nc.sync.dma_start(
                        out[s].rearrange("(t p) d -> p t d", p=P)[:, 4 * c : 4 * c + 4, :],
                        dd[:],
                    )

            def emit_qk(kf, qf, ih, jp):
                """QK^T for one j-pair: 4 paired bf16 matmuls on disjoint PE
                row groups (0-63 / 64-127 concurrent) + exp per chunk — on
                ACT (exact), or on DVE via Schraudolph for DVE_SET chunks."""
                es = espool.tile([P, 2048], bf16, tag="es")
                srcs = [None, None]
                for c in range(2):
                    st = pstpool.tile([P, 1024], f32, tag="pst")
                    nc.tensor.matmul(
                        st[:, 0:512],
                        kf[0:D, jp, :],
                        qf[0:D, 1024 * ih + 512 * c : 1024 * ih + 512 * (c + 1)],
                        start=True,
                        stop=True,
                    )
                    nc.tensor.matmul(
                        st[:, 512:1024],
                        kf[D:P, jp, :],
                        qf[D:P, 1024 * ih + 512 * c : 1024 * ih + 512 * (c + 1)],
                        start=True,
                        stop=True,
                    )
                    if (jp, c) in DVE_SET:
                        esd = esdpool.tile([P, 1024], i16, tag="esd")
                        nc.vector.tensor_scalar(
                            esd[:], st[:], A_EFF, B_SCH, ALU.mult, ALU.add
                        )
                        srcs[c] = esd[:].bitcast(bf16)
                    else:
                        nc.scalar.activation(
                            es[:, 1024 * c : 1024 * (c + 1)],
                            st[:],
                            AF.Exp,
                            scale=1.0 / 8.0,
                        )
                        srcs[c] = es[:, 1024 * c : 1024 * (c + 1)]
                return srcs

            def emit_half(s_rep, kf, qf, vf, ih, onn, fillers, pending,
                          tail=None):
                """One query-half loop over j-pairs starting at jp1 — jp0's
                QK^T/exp was emitted inside the previous half (or prologue)
                and arrives via `pending`. The PREVIOUS pair's 4 PV matmuls
                lag the exp by one step; `fillers` are emitted one per step.
                `tail` (emitted between the last exp and the last PV) emits
                the NEXT half's jp0 QK^T/exp so the ACT stream never breaks
                at half boundaries."""
                ot2 = [
                    potpool.tile([D + 1, 512], f32, tag="pot", name=f"ot_{ih}_{c}")
                    for c in range(2)
                ]

                def emit_pv(jp, srcs):
                    j0, j1 = 2 * jp, 2 * jp + 1
                    first = jp == 0
                    last = jp == NP - 1
                    for c in range(2):
                        nc.tensor.matmul(
                            ot2[c][:],
                            vf[:, j0, :],
                            srcs[c][:, 0:512],
                            start=first,
                            stop=False,
                        )
                        nc.tensor.matmul(
                            ot2[c][:],
                            vf[:, j1, :],
                            srcs[c][:, 512:1024],
                            start=False,
                            stop=last,
                        )

                for jp in range(1, NP):
                    srcs = emit_qk(kf, qf, ih, jp)
                    emit_pv(*pending)
                    pending = (jp, srcs)
                    if fillers:
                        fillers.pop(0)()
                if tail is not None:
                    tail()
                emit_pv(*pending)
                return [
                    (lambda c=c: emit_chunk_epilogue(s_rep, ih, c, ot2, onn))
                    for c in range(2)
                ]

            # ---- software-pipelined emission across slabs AND iterations ---
            # Prologue (once): slab0 loads + lambda + slab0's first QK^T/exp.
            # Body: rotated modulo the iteration — slab0-ih0 starts at jp1
            # consuming the pending pair; the tail emits the NEXT iteration's
            # first QK^T/exp (into the same ring slots as the prologue's, so
            # addresses are iteration-invariant) BEFORE draining the last
            # epilogues, keeping ACT fed across the loop-wrap.
            def emit_body(last):
                def make_tail(tiles_ref, ih):
                    def tail():
                        kfn, qfn, _ = tiles_ref[0]
                        pend_qk[0] = (0, emit_qk(kfn, qfn, ih, 0))

                    return tail

                kf, qf, vf = tiles[0]
                cur = [(kf, qf, vf)]
                onn0 = osbpool.tile([P, NT, D], f32, tag="onn")
                epi0 = emit_half(0, kf, qf, vf, 0, onn0, [], pend_qk[0],
                                 tail=make_tail(cur, 1))
                tiles[0] = emit_loads(1)
                epi1 = emit_half(0, kf, qf, vf, 1, onn0, epi0, pend_qk[0],
                                 tail=make_tail(tiles, 0))
                kf1, qf1, vf1 = tiles[0]
                cur1 = [(kf1, qf1, vf1)]
                onn1 = osbpool.tile([P, NT, D], f32, tag="onn")
                epi0b = emit_half(1, kf1, qf1, vf1, 0, onn1, epi1, pend_qk[0],
                                  tail=make_tail(cur1, 1))
                tiles[0] = emit_loads(2)
                epi1b = emit_half(1, kf1, qf1, vf1, 1, onn1, epi0b, pend_qk[0],
                                  tail=None if last else make_tail(tiles, 0))
                for thunk in epi1b:
                    thunk()

            tiles = [emit_loads(0)]
            emit_lambda()
            kf0, qf0, vf0 = tiles[0]
            pend_qk = [(0, emit_qk(kf0, qf0, 0, 0))]
            if repeats == 1:
                emit_body(last=True)
            else:
                with tc.For_i(0, repeats, 1):
                    emit_body(last=False)

    nc.compile()
    _cached_nc[repeats] = nc
    return nc


def staged_in_maps(inputs):
    """FULL host inputs -> per-core input dicts (the sharding + layouts)."""
    import ml_dtypes

    bf16 = ml_dtypes.bfloat16

    q = np.asarray(inputs["q"], dtype=np.float32)
    k = np.asarray(inputs["k"], dtype=np.float32)
    v = np.asarray(inputs["v"], dtype=np.float32)
    lams = np.concatenate(
        [
            np.asarray(inputs["lambda_q1"], dtype=np.float32),
            np.asarray(inputs["lambda_k1"], dtype=np.float32),
            np.asarray(inputs["lambda_q2"], dtype=np.float32),
            np.asarray(inputs["lambda_k2"], dtype=np.float32),
        ]
    ).reshape(1, 4 * D)
    # tiled to all partitions so lambda is computed per-partition on device
    lams = np.ascontiguousarray(np.broadcast_to(lams, (P, 4 * D)))

    # [b, n, h, d] -> slabs, b-major
    qT = q.transpose(0, 2, 3, 1).reshape(B * H, D, N).astype(bf16)
    kT = k.transpose(0, 2, 3, 1).reshape(B * H, D, N).astype(bf16)
    vso = v.transpose(0, 2, 1, 3).reshape(B * H, N, D).astype(bf16)

    # q^T duplicated into both partition halves: [S, 128, N]
    qs = np.concatenate([qT, qT], axis=1)
    # k^T row-packed: even j-tiles on partitions 0-63, odd on 64-127
    kt3 = kT.reshape(B * H, D, NT, P)
    ks = np.concatenate([kt3[:, :, 0::2, :], kt3[:, :, 1::2, :]], axis=1)
    # V'' = [V | 1] in [S, 128, NT, 65] tile layout
    vt = vso.reshape(B * H, NT, P, D).transpose(0, 2, 1, 3)
    ones = np.ones((B * H, P, NT, 1), dtype=bf16)
    vsp = np.concatenate([vt, ones], axis=3)

    return [
        {
            "qs": np.ascontiguousarray(qs[SLABS_PER_CORE * c : SLABS_PER_CORE * (c + 1)]),
            "ks": np.ascontiguousarray(ks[SLABS_PER_CORE * c : SLABS_PER_CORE * (c + 1)]),
            "vs": np.ascontiguousarray(vsp[SLABS_PER_CORE * c : SLABS_PER_CORE * (c + 1)]),
            "lams": lams,
        }
        for c in range(N_CORES)
    ]


def unshard_out(stacked):
    """[n_cores, *per-core out shape] -> [b, h, n/2, d] fp32."""
    return np.asarray(stacked).reshape(B, H, N // 2, D).astype(np.float32)


def kernel(q, k, v, lambda_q1, lambda_k1, lambda_q2, lambda_k2, **_unused):
    from concourse.bass_utils import run_bass_kernel_spmd

    in_maps = staged_in_maps(
        dict(q=q, k=k, v=v, lambda_q1=lambda_q1, lambda_k1=lambda_k1,
             lambda_q2=lambda_q2, lambda_k2=lambda_k2)
    )
    nc = _build_program()
    res = run_bass_kernel_spmd(nc, in_maps, core_ids=list(range(N_CORES)))
    outs = np.stack([res.results[c]["out"] for c in range(N_CORES)])
    return unshard_out(outs)
